# revision 7
# baseline (speedup 1.0000x reference)
"""Distributed GCN (3x GCNConv + global mean pool + linear) on 8 TRN2 cores.

Sharding: nodes partitioned contiguously across 8 cores; edges partitioned by
dst owner; per-layer node features all-gathered to a full replicated table in
each core's DRAM; per-edge messages fetched with dma_gather; segment-sum over
dst done as one-hot matmuls accumulating in PSUM per 128-node dst window.
"""
import math
import numpy as np
from contextlib import ExitStack

import concourse.bacc as bacc
import concourse.mybir as mybir
from concourse.tile import TileContext
from concourse.bass_utils import run_bass_kernel_spmd
from concourse.masks import make_identity

P = 128
NCORES = 8
N = 100000
E = 1600000
H = 128
C = 10
G = 128
NP = N // NCORES            # 12500 nodes per core
NW = math.ceil(NP / P)      # 98 dst windows per core
NPAD = NW * P               # 12544 padded nodes per core
CHROWS = 32768              # gather chunk rows (int16 index limit)
NCH = math.ceil(N / CHROWS)  # 4
NI = 1024                   # indices per dma_gather call (>1024 crashes SWDGE ring)
NLAYERS = 3                 # debug knob

TRACE = False               # set by test.py for profiling runs
LAST_RESULTS = {}           # debug: per-core raw results


def _wrap_idx(idx):
    """int16 gather index layout: [128, len/16], i -> [i%16, i//16], tiled x8."""
    n = idx.shape[0]
    assert n % 16 == 0
    w = idx.reshape(n // 16, 16).T.astype(np.int16)   # [16, n/16]
    return np.tile(w, (8, 1))                          # [128, n/16]


def _preprocess(edge_index):
    """Partition/sort/pad edges. Returns shared structure + per-core arrays."""
    src = np.asarray(edge_index[0], dtype=np.int64)
    dst = np.asarray(edge_index[1], dtype=np.int64)

    deg = np.bincount(dst, minlength=N).astype(np.float32) + 1.0
    dinv = (1.0 / np.sqrt(deg)).astype(np.float32)

    # self loops
    loops = np.arange(N, dtype=np.int64)
    src_a = np.concatenate([src, loops])
    dst_a = np.concatenate([dst, loops])

    owner = dst_a // NP
    # per-core bucket counts: cnt[core][w][ch]
    w_all = (dst_a - owner * NP) // P
    ch_all = src_a // CHROWS
    flat = ((owner * NW + w_all) * NCH + ch_all).astype(np.int64)
    cnt = np.bincount(flat, minlength=NCORES * NW * NCH).reshape(NCORES, NW, NCH)
    ngrp = np.maximum(np.ceil(cnt.max(axis=0) / P).astype(np.int64), 0)  # [NW, NCH]

    cores = []
    order_all = np.argsort(flat, kind="stable")
    bounds = np.searchsorted(flat[order_all], np.arange(NCORES * NW * NCH + 1))
    for c in range(NCORES):
        idx_parts = [[] for _ in range(NCH)]   # per-chunk local row ids
        slot_parts = []                        # per-group 128 dst slots (f32)
        for w in range(NW):
            for ch in range(NCH):
                b = (c * NW + w) * NCH + ch
                ee = order_all[bounds[b]:bounds[b + 1]]
                k = ee.shape[0]
                npad = int(ngrp[w, ch]) * P - k
                loc = np.zeros(int(ngrp[w, ch]) * P, dtype=np.int64)
                slo = np.full(int(ngrp[w, ch]) * P, -1.0, dtype=np.float32)
                loc[:k] = src_a[ee] - ch * CHROWS
                slo[:k] = (dst_a[ee] - c * NP - w * P).astype(np.float32)
                idx_parts[ch].append(loc)
                slot_parts.append(slo)
        widx = np.concatenate(
            [_wrap_idx(np.concatenate(p)) if p else np.zeros((128, 0), np.int16)
             for p in idx_parts], axis=1)
        dstslot = np.concatenate(slot_parts).reshape(-1, P).T.copy()  # [128, NG]
        cores.append((widx, dstslot))
    return ngrp, dinv, deg, cores


def _build(ngrp, has_bias, has_blin):
    """Build the SPMD bass program (same for all cores)."""
    nc = bacc.Bacc("TRN2", num_devices=NCORES)
    f32 = mybir.dt.float32

    # chunk stream lengths / call table
    L = [int(ngrp[:, ch].sum()) * P for ch in range(NCH)]
    Loff = np.concatenate([[0], np.cumsum(L)])
    NG = int(ngrp.sum())

    # ---- I/O ----
    xT = nc.dram_tensor("xT", [P, NPAD], f32, kind="ExternalInput")
    widx = nc.dram_tensor("widx", [P, Loff[-1] // 16], mybir.dt.int16,
                          kind="ExternalInput")
    dstslot_d = nc.dram_tensor("dstslot", [P, NG], f32, kind="ExternalInput")
    dinvw_d = nc.dram_tensor("dinvw", [P, NW], f32, kind="ExternalInput")
    batchslot_d = nc.dram_tensor("batchslot", [P, NW], f32, kind="ExternalInput")
    cntinv_d = nc.dram_tensor("cntinv", [P, 1], f32, kind="ExternalInput")
    Ws_d = [nc.dram_tensor(f"W{i}", [P, H], f32, kind="ExternalInput")
            for i in range(3)]
    Wlin_d = nc.dram_tensor("Wlin", [P, C], f32, kind="ExternalInput")
    sqdegw_d = bfull_d = None
    if has_bias:
        sqdegw_d = nc.dram_tensor("sqdegw", [P, NW], f32, kind="ExternalInput")
        bfull_d = [nc.dram_tensor(f"Bfull{i}", [P, H], f32, kind="ExternalInput")
                   for i in range(3)]
    blin_d = None
    if has_blin:
        blin_d = nc.dram_tensor("blinT", [P, C], f32, kind="ExternalInput")
    out_d = nc.dram_tensor("out", [G, C], f32, kind="ExternalOutput")

    y_local = nc.dram_tensor("y_local", [NP, H], f32, kind="Internal")
    y_full = nc.dram_tensor("y_full", [N, H], f32, kind="Internal",
                            addr_space="Shared")
    ar_in = nc.dram_tensor("ar_in", [G, H], f32, kind="Internal")
    ar_out = nc.dram_tensor("ar_out", [G, H], f32, kind="Internal",
                            addr_space="Shared")

    with TileContext(nc) as tc:
        with ExitStack() as ctx:
            pers = ctx.enter_context(tc.tile_pool(name="pers", bufs=1))
            sy = ctx.enter_context(tc.tile_pool(name="sy", bufs=3))
            soh = ctx.enter_context(tc.tile_pool(name="soh", bufs=6))
            sep = ctx.enter_context(tc.tile_pool(name="sep", bufs=3))
            sidx = ctx.enter_context(tc.tile_pool(name="sidx", bufs=4))
            gpools = [ctx.enter_context(tc.tile_pool(name=f"gat{ch}", bufs=2))
                      for ch in range(NCH)]
            psy = ctx.enter_context(tc.tile_pool(name="psy", bufs=2, space="PSUM"))
            psa = ctx.enter_context(tc.tile_pool(name="psa", bufs=2, space="PSUM"))
            pst = ctx.enter_context(tc.tile_pool(name="pst", bufs=2, space="PSUM"))
            psp = ctx.enter_context(tc.tile_pool(name="psp", bufs=1, space="PSUM"))

            # ---- persistent tiles ----
            hT = pers.tile([P, NPAD], f32)
            nc.sync.dma_start(out=hT[:], in_=xT[:])
            dstslot = pers.tile([P, NG], f32)
            nc.sync.dma_start(out=dstslot[:], in_=dstslot_d[:])
            dinvw = pers.tile([P, NW], f32)
            nc.sync.dma_start(out=dinvw[:], in_=dinvw_d[:])
            batchslot = pers.tile([P, NW], f32)
            nc.sync.dma_start(out=batchslot[:], in_=batchslot_d[:])
            cntinv = pers.tile([P, 1], f32)
            nc.sync.dma_start(out=cntinv[:], in_=cntinv_d[:])
            Ws = []
            for i in range(3):
                t = pers.tile([P, H], f32, tag=f"W{i}")
                nc.sync.dma_start(out=t[:], in_=Ws_d[i][:])
                Ws.append(t)
            Wlin = pers.tile([P, C], f32)
            nc.sync.dma_start(out=Wlin[:], in_=Wlin_d[:])
            sqdegw = bfull = blin = None
            if has_bias:
                sqdegw = pers.tile([P, NW], f32)
                nc.sync.dma_start(out=sqdegw[:], in_=sqdegw_d[:])
                bfull = []
                for i in range(3):
                    t = pers.tile([P, H], f32, tag=f"Bf{i}")
                    nc.sync.dma_start(out=t[:], in_=bfull_d[i][:])
                    bfull.append(t)
            if has_blin:
                blin = pers.tile([P, C], f32)
                nc.sync.dma_start(out=blin[:], in_=blin_d[:])

            ident = pers.tile([P, P], f32)
            make_identity(nc, ident[:])
            iota_i = pers.tile([P, P], mybir.dt.int32)
            nc.gpsimd.iota(iota_i[:], pattern=[[1, P]], base=0,
                           channel_multiplier=0)
            iota_f = pers.tile([P, P], f32)
            nc.vector.tensor_copy(out=iota_f[:], in_=iota_i[:])

            pool_acc = pers.tile([P, H], f32)   # pooled [graph, feat] accum
            nc.vector.memset(pool_acc[:], 0.0)

            # gather call schedule per chunk: list of (start, n) within chunk
            calls = []
            for ch in range(NCH):
                cs = []
                p = 0
                while p < L[ch]:
                    n = min(NI, L[ch] - p)
                    cs.append((p, n))
                    p += n
                calls.append(cs)

            for layer in range(NLAYERS):
                # ---- y = (h @ W) * dinv per window -> y_local -> allgather
                for w in range(NW):
                    py = psy.tile([P, H], f32, space="PSUM", tag="py")
                    nc.tensor.matmul(out=py[:], lhsT=hT[:, w * P:(w + 1) * P],
                                     rhs=Ws[layer][:], start=True, stop=True)
                    yt = sy.tile([P, H], f32, tag="yt")
                    nc.vector.tensor_scalar(
                        out=yt[:], in0=py[:], scalar1=dinvw[:, w:w + 1],
                        scalar2=None, op0=mybir.AluOpType.mult)
                    rows = min(NP - w * P, P)
                    nc.sync.dma_start(out=y_local[w * P:w * P + rows, :],
                                      in_=yt[:rows, :])
                nc.gpsimd.collective_compute(
                    "AllGather", mybir.AluOpType.bypass,
                    ins=[y_local[:]], outs=[y_full[:]],
                    replica_groups=[list(range(NCORES))],
                )

                # ---- edge gather + segment-sum matmuls ----
                # stream state per chunk
                cur = [-1] * NCH          # current call index per chunk
                gtile = [None] * NCH
                pos = [0] * NCH           # consumed edges within chunk

                def next_group(ch):
                    if cur[ch] < 0 or pos[ch] >= calls[ch][cur[ch]][0] + calls[ch][cur[ch]][1]:
                        cur[ch] += 1
                        start, n = calls[ch][cur[ch]]
                        it = sidx.tile([P, NI // 16], mybir.dt.int16, tag="idx")
                        c0 = (Loff[ch] + start) // 16
                        nc.sync.dma_start(out=it[:, :n // 16],
                                          in_=widx[:, c0:c0 + n // 16])
                        gt = gpools[ch].tile([P, NI // P, H], f32, tag=f"g{ch}")
                        rows0 = ch * CHROWS
                        rows1 = min(rows0 + CHROWS, N)
                        nc.gpsimd.dma_gather(
                            out_ap=gt[:, :n // P, :],
                            in_ap=y_full[rows0:rows1],
                            idxs_ap=it[:, :n // 16],
                            num_idxs=n,
                            num_idxs_reg=n,
                            elem_size=H,
                        )
                        gtile[ch] = gt
                    start, _ = calls[ch][cur[ch]]
                    t = (pos[ch] - start) // P
                    pos[ch] += P
                    return gtile[ch][:, t, :]

                gcol = 0
                for w in range(NW):
                    pa = psa.tile([P, H], f32, space="PSUM", tag="pa")
                    first = True
                    if has_bias:
                        diag = soh.tile([P, P], f32, tag="diag")
                        nc.gpsimd.affine_select(
                            out=diag[:],
                            in_=sqdegw[:, w:w + 1].to_broadcast([P, P]),
                            pattern=[[-1, P]], base=0, channel_multiplier=1,
                            compare_op=mybir.AluOpType.is_equal, fill=0.0)
                        nc.tensor.matmul(out=pa[:], lhsT=diag[:],
                                         rhs=bfull[layer][:],
                                         start=True, stop=False)
                        first = False
                    ng_w = int(ngrp[w].sum())
                    done = 0
                    for ch in range(NCH):
                        for g in range(int(ngrp[w, ch])):
                            rhs = next_group(ch)
                            oh = soh.tile([P, P], f32, tag="oh")
                            nc.any.tensor_scalar(
                                out=oh[:], in0=iota_f[:],
                                scalar1=dstslot[:, gcol:gcol + 1],
                                scalar2=None,
                                op0=mybir.AluOpType.is_equal)
                            gcol += 1
                            done += 1
                            nc.tensor.matmul(
                                out=pa[:], lhsT=oh[:], rhs=rhs,
                                start=first, stop=(done == ng_w))
                            first = False

                    # ---- epilogue: relu(dinv * pa) ----
                    s3 = sep.tile([P, H], f32, tag="s3")
                    nc.scalar.activation(
                        out=s3[:], in_=pa[:],
                        func=mybir.ActivationFunctionType.Relu,
                        scale=dinvw[:, w:w + 1])
                    if layer < 2:
                        pt = pst.tile([P, P], f32, space="PSUM", tag="pt")
                        nc.tensor.transpose(out=pt[:], in_=s3[:],
                                            identity=ident[:])
                        nc.vector.tensor_copy(out=hT[:, w * P:(w + 1) * P],
                                              in_=pt[:])
                    else:
                        ohb = soh.tile([P, P], f32, tag="oh")
                        nc.any.tensor_scalar(
                            out=ohb[:], in0=iota_f[:],
                            scalar1=batchslot[:, w:w + 1], scalar2=None,
                            op0=mybir.AluOpType.is_equal)
                        pp = psp.tile([P, H], f32, space="PSUM", tag="pp")
                        nc.tensor.matmul(out=pp[:], lhsT=ohb[:], rhs=s3[:],
                                         start=True, stop=True)
                        if w == 0:
                            nc.vector.tensor_copy(out=pool_acc[:], in_=pp[:])
                        else:
                            nc.vector.tensor_tensor(
                                out=pool_acc[:], in0=pool_acc[:], in1=pp[:],
                                op=mybir.AluOpType.add)

            # ---- pooling finish ----
            nc.sync.dma_start(out=ar_in[:], in_=pool_acc[:])
            nc.gpsimd.collective_compute(
                "AllReduce", mybir.AluOpType.add,
                ins=[ar_in[:]], outs=[ar_out[:]],
                replica_groups=[list(range(NCORES))],
            )
            art = sep.tile([P, H], f32, tag="art")
            nc.sync.dma_start(out=art[:], in_=ar_out[:])
            ptile = sep.tile([P, H], f32, tag="ptile")
            nc.vector.tensor_scalar(
                out=ptile[:], in0=art[:], scalar1=cntinv[:, 0:1],
                scalar2=None, op0=mybir.AluOpType.mult)
            ptp = pst.tile([P, P], f32, space="PSUM", tag="pt")
            nc.tensor.transpose(out=ptp[:], in_=ptile[:], identity=ident[:])
            ptT = sep.tile([P, P], f32, tag="ptT")
            nc.vector.tensor_copy(out=ptT[:], in_=ptp[:])
            po = pst.tile([P, P], f32, space="PSUM", tag="pt")
            nc.tensor.matmul(out=po[:, :C], lhsT=ptT[:], rhs=Wlin[:],
                             start=True, stop=True)
            ot = sep.tile([P, C], f32, tag="ot")
            if has_blin:
                nc.vector.tensor_tensor(out=ot[:], in0=po[:, :C], in1=blin[:],
                                        op=mybir.AluOpType.add)
            else:
                nc.vector.tensor_copy(out=ot[:], in_=po[:, :C])
            nc.sync.dma_start(out=out_d[:], in_=ot[:])

    nc.compile()
    return nc


def kernel(x, edge_index, batch, W0, b0, W1, b1, W2, b2, Wlin, blin):
    x = np.asarray(x, dtype=np.float32)
    batch_np = np.asarray(batch, dtype=np.int64)
    Wl = [np.asarray(w, dtype=np.float32) for w in (W0, W1, W2)]
    bl = [np.asarray(b, dtype=np.float32) for b in (b0, b1, b2)]
    Wlin = np.asarray(Wlin, dtype=np.float32)
    blin = np.asarray(blin, dtype=np.float32)

    ngrp, dinv, deg, cores = _preprocess(np.asarray(edge_index))
    has_bias = any(np.abs(b).max() > 0 for b in bl)
    has_blin = bool(np.abs(blin).max() > 0)

    cnt = np.bincount(batch_np, minlength=G).astype(np.float32)
    cntinv = (1.0 / np.maximum(cnt, 1.0)).astype(np.float32)[:, None]  # [G,1]

    in_maps = []
    for c in range(NCORES):
        widx, dstslot = cores[c]
        lo = c * NP
        xT = np.zeros((P, NPAD), dtype=np.float32)
        xT[:, :NP] = x[lo:lo + NP].T
        dv = np.ones(NPAD, dtype=np.float32)
        dv[:NP] = dinv[lo:lo + NP]
        dinvw = dv.reshape(NW, P).T.copy()
        bs = np.full(NPAD, -1.0, dtype=np.float32)
        bs[:NP] = batch_np[lo:lo + NP].astype(np.float32)
        batchslot = bs.reshape(NW, P).T.copy()
        m = {
            "xT": xT, "widx": widx, "dstslot": dstslot,
            "dinvw": dinvw, "batchslot": batchslot, "cntinv": cntinv,
            "W0": Wl[0], "W1": Wl[1], "W2": Wl[2], "Wlin": Wlin,
        }
        if has_bias:
            sq = np.zeros(NPAD, dtype=np.float32)
            sq[:NP] = np.sqrt(deg[lo:lo + NP])
            m["sqdegw"] = sq.reshape(NW, P).T.copy()
            for i in range(3):
                m[f"Bfull{i}"] = np.tile(bl[i][None, :], (P, 1)).astype(np.float32)
        if has_blin:
            m["blinT"] = np.tile(blin[None, :], (P, 1)).astype(np.float32)
        in_maps.append(m)

    nc = _build(ngrp, has_bias, has_blin)
    res = run_bass_kernel_spmd(nc, in_maps, core_ids=list(range(NCORES)),
                               trace=TRACE)
    global LAST_RESULTS
    LAST_RESULTS = res
    return res.results[0]["out"]


# revision 8
# speedup vs baseline: 5.8208x; 5.8208x over previous
"""Distributed GCN (3x GCNConv + global mean pool + linear) on 8 TRN2 cores.

Sharding: nodes partitioned contiguously across 8 cores; edges partitioned by
dst owner; per-layer node features all-gathered to a full replicated table in
each core's DRAM; per-edge messages fetched with dma_gather; segment-sum over
dst done as one-hot matmuls accumulating in PSUM per 128-node dst window.
"""
import math
import numpy as np
from contextlib import ExitStack

import concourse.bacc as bacc
import concourse.mybir as mybir
from concourse.tile import TileContext
from concourse.bass_utils import run_bass_kernel_spmd
from concourse.masks import make_identity

P = 128
NCORES = 8
N = 100000
E = 1600000
H = 128
C = 10
G = 128
NP = N // NCORES            # 12500 nodes per core
NW = math.ceil(NP / P)      # 98 dst windows per core
NPAD = NW * P               # 12544 padded nodes per core
CHROWS = 25000              # gather chunk rows (int16 limit 32767; 25000 balances buckets)
NCH = math.ceil(N / CHROWS)  # 4
NI = 1024                   # indices per dma_gather call (>1024 crashes SWDGE ring)
NLAYERS = 3                 # debug knob

TRACE = False               # set by test.py for profiling runs
LAST_RESULTS = {}           # debug: per-core raw results


def _wrap_idx(idx):
    """int16 gather index layout: [128, len/16], i -> [i%16, i//16], tiled x8."""
    n = idx.shape[0]
    assert n % 16 == 0
    w = idx.reshape(n // 16, 16).T.astype(np.int16)   # [16, n/16]
    return np.tile(w, (8, 1))                          # [128, n/16]


def _preprocess(edge_index):
    """Partition/sort/pad edges. Returns shared structure + per-core arrays."""
    src = np.asarray(edge_index[0], dtype=np.int64)
    dst = np.asarray(edge_index[1], dtype=np.int64)

    deg = np.bincount(dst, minlength=N).astype(np.float32) + 1.0
    dinv = (1.0 / np.sqrt(deg)).astype(np.float32)

    # self-loop term handled on-device via the ySB slab, not as edges
    src_a = src
    dst_a = dst

    owner = dst_a // NP
    # per-core bucket counts: cnt[core][w][ch]
    w_all = (dst_a - owner * NP) // P
    ch_all = src_a // CHROWS
    flat = ((owner * NW + w_all) * NCH + ch_all).astype(np.int64)
    cnt = np.bincount(flat, minlength=NCORES * NW * NCH).reshape(NCORES, NW, NCH)
    ngrp = np.maximum(np.ceil(cnt.max(axis=0) / P).astype(np.int64), 0)  # [NW, NCH]

    cores = []
    order_all = np.argsort(flat, kind="stable")
    bounds = np.searchsorted(flat[order_all], np.arange(NCORES * NW * NCH + 1))
    for c in range(NCORES):
        idx_parts = [[] for _ in range(NCH)]   # per-chunk local row ids
        slot_parts = []                        # per-group 128 dst slots (f32)
        for w in range(NW):
            for ch in range(NCH):
                b = (c * NW + w) * NCH + ch
                ee = order_all[bounds[b]:bounds[b + 1]]
                k = ee.shape[0]
                npad = int(ngrp[w, ch]) * P - k
                loc = np.zeros(int(ngrp[w, ch]) * P, dtype=np.int64)
                slo = np.full(int(ngrp[w, ch]) * P, -1.0, dtype=np.float32)
                loc[:k] = src_a[ee] - ch * CHROWS
                slo[:k] = (dst_a[ee] - c * NP - w * P).astype(np.float32)
                idx_parts[ch].append(loc)
                slot_parts.append(slo)
        widx = np.concatenate(
            [_wrap_idx(np.concatenate(p)) if p else np.zeros((128, 0), np.int16)
             for p in idx_parts], axis=1)
        dstslot = np.concatenate(slot_parts).reshape(-1, P).T.copy()  # [128, NG]
        cores.append((widx, dstslot))
    return ngrp, dinv, deg, cores


def _build(ngrp, has_bias, has_blin):
    """Build the SPMD bass program (same for all cores)."""
    nc = bacc.Bacc("TRN2", num_devices=NCORES)
    f32 = mybir.dt.float32

    # chunk stream lengths / call table
    L = [int(ngrp[:, ch].sum()) * P for ch in range(NCH)]
    Loff = np.concatenate([[0], np.cumsum(L)])
    NG = int(ngrp.sum())

    # ---- I/O ----
    xT = nc.dram_tensor("xT", [P, NPAD], f32, kind="ExternalInput")
    widx = nc.dram_tensor("widx", [P, Loff[-1] // 16], mybir.dt.int16,
                          kind="ExternalInput")
    dstslot_d = nc.dram_tensor("dstslot", [P, NG], f32, kind="ExternalInput")
    dinvw_d = nc.dram_tensor("dinvw", [P, NW], f32, kind="ExternalInput")
    batchslot_d = nc.dram_tensor("batchslot", [P, NW], f32, kind="ExternalInput")
    cntinv_d = nc.dram_tensor("cntinv", [P, 1], f32, kind="ExternalInput")
    Ws_d = [nc.dram_tensor(f"W{i}", [P, H], f32, kind="ExternalInput")
            for i in range(3)]
    Wlin_d = nc.dram_tensor("Wlin", [P, C], f32, kind="ExternalInput")
    sqdegw_d = bfull_d = None
    if has_bias:
        sqdegw_d = nc.dram_tensor("sqdegw", [P, NW], f32, kind="ExternalInput")
        bfull_d = [nc.dram_tensor(f"Bfull{i}", [P, H], f32, kind="ExternalInput")
                   for i in range(3)]
    blin_d = None
    if has_blin:
        blin_d = nc.dram_tensor("blinT", [P, C], f32, kind="ExternalInput")
    out_d = nc.dram_tensor("out", [G, C], f32, kind="ExternalOutput")

    y_local = nc.dram_tensor("y_local", [NP, H], f32, kind="Internal")
    y_full = nc.dram_tensor("y_full", [N, H], f32, kind="Internal",
                            addr_space="Shared")
    ar_in = nc.dram_tensor("ar_in", [G, H], f32, kind="Internal")
    ar_out = nc.dram_tensor("ar_out", [G, H], f32, kind="Internal",
                            addr_space="Shared")

    with TileContext(nc) as tc:
        with ExitStack() as ctx:
            pers = ctx.enter_context(tc.tile_pool(name="pers", bufs=1))
            sy = ctx.enter_context(tc.tile_pool(name="sy", bufs=3))
            soh = ctx.enter_context(tc.tile_pool(name="soh", bufs=6))
            sep = ctx.enter_context(tc.tile_pool(name="sep", bufs=3))
            sidx = ctx.enter_context(tc.tile_pool(name="sidx", bufs=4))
            gpools = [ctx.enter_context(tc.tile_pool(name=f"gat{ch}", bufs=2))
                      for ch in range(NCH)]
            psy = ctx.enter_context(tc.tile_pool(name="psy", bufs=2, space="PSUM"))
            psa = ctx.enter_context(tc.tile_pool(name="psa", bufs=2, space="PSUM"))
            pst = ctx.enter_context(tc.tile_pool(name="pst", bufs=2, space="PSUM"))
            psp = ctx.enter_context(tc.tile_pool(name="psp", bufs=1, space="PSUM"))

            # ---- persistent tiles ----
            hT = pers.tile([P, NPAD], f32)
            nc.sync.dma_start(out=hT[:], in_=xT[:])
            ySB = pers.tile([P, NPAD], f32)
            dstslot = pers.tile([P, NG], f32)
            nc.sync.dma_start(out=dstslot[:], in_=dstslot_d[:])
            dinvw = pers.tile([P, NW], f32)
            nc.sync.dma_start(out=dinvw[:], in_=dinvw_d[:])
            batchslot = pers.tile([P, NW], f32)
            nc.sync.dma_start(out=batchslot[:], in_=batchslot_d[:])
            cntinv = pers.tile([P, 1], f32)
            nc.sync.dma_start(out=cntinv[:], in_=cntinv_d[:])
            Ws = []
            for i in range(3):
                t = pers.tile([P, H], f32, tag=f"W{i}")
                nc.sync.dma_start(out=t[:], in_=Ws_d[i][:])
                Ws.append(t)
            Wlin = pers.tile([P, C], f32)
            nc.sync.dma_start(out=Wlin[:], in_=Wlin_d[:])
            sqdegw = bfull = blin = None
            if has_bias:
                sqdegw = pers.tile([P, NW], f32)
                nc.sync.dma_start(out=sqdegw[:], in_=sqdegw_d[:])
                bfull = []
                for i in range(3):
                    t = pers.tile([P, H], f32, tag=f"Bf{i}")
                    nc.sync.dma_start(out=t[:], in_=bfull_d[i][:])
                    bfull.append(t)
            if has_blin:
                blin = pers.tile([P, C], f32)
                nc.sync.dma_start(out=blin[:], in_=blin_d[:])

            ident = pers.tile([P, P], f32)
            make_identity(nc, ident[:])
            iota_i = pers.tile([P, P], mybir.dt.int32)
            nc.gpsimd.iota(iota_i[:], pattern=[[1, P]], base=0,
                           channel_multiplier=0)
            iota_f = pers.tile([P, P], f32)
            nc.vector.tensor_copy(out=iota_f[:], in_=iota_i[:])

            pool_acc = pers.tile([P, H], f32)   # pooled [graph, feat] accum
            nc.vector.memset(pool_acc[:], 0.0)

            # gather call schedule per chunk: list of (start, n) within chunk
            calls = []
            for ch in range(NCH):
                cs = []
                p = 0
                while p < L[ch]:
                    n = min(NI, L[ch] - p)
                    cs.append((p, n))
                    p += n
                calls.append(cs)

            for layer in range(NLAYERS):
                # ---- y = (h @ W) * dinv per window -> y_local -> allgather
                for w in range(NW):
                    py = psy.tile([P, H], f32, space="PSUM", tag="py")
                    nc.tensor.matmul(out=py[:], lhsT=hT[:, w * P:(w + 1) * P],
                                     rhs=Ws[layer][:], start=True, stop=True)
                    nc.vector.tensor_scalar(
                        out=ySB[:, w * P:(w + 1) * P], in0=py[:],
                        scalar1=dinvw[:, w:w + 1],
                        scalar2=None, op0=mybir.AluOpType.mult)
                    rows = min(NP - w * P, P)
                    nc.sync.dma_start(out=y_local[w * P:w * P + rows, :],
                                      in_=ySB[:rows, w * P:(w + 1) * P])
                nc.gpsimd.collective_compute(
                    "AllGather", mybir.AluOpType.bypass,
                    ins=[y_local[:]], outs=[y_full[:]],
                    replica_groups=[list(range(NCORES))],
                )

                # ---- edge gather + segment-sum matmuls ----
                # stream state per chunk
                cur = [-1] * NCH          # current call index per chunk
                gtile = [None] * NCH
                pos = [0] * NCH           # consumed edges within chunk

                def next_group(ch):
                    if cur[ch] < 0 or pos[ch] >= calls[ch][cur[ch]][0] + calls[ch][cur[ch]][1]:
                        cur[ch] += 1
                        start, n = calls[ch][cur[ch]]
                        it = sidx.tile([P, NI // 16], mybir.dt.int16, tag="idx")
                        c0 = (Loff[ch] + start) // 16
                        nc.sync.dma_start(out=it[:, :n // 16],
                                          in_=widx[:, c0:c0 + n // 16])
                        gt = gpools[ch].tile([P, NI // P, H], f32, tag=f"g{ch}")
                        rows0 = ch * CHROWS
                        rows1 = min(rows0 + CHROWS, N)
                        nc.gpsimd.dma_gather(
                            out_ap=gt[:, :n // P, :],
                            in_ap=y_full[rows0:rows1],
                            idxs_ap=it[:, :n // 16],
                            num_idxs=n,
                            num_idxs_reg=n,
                            elem_size=H,
                        )
                        gtile[ch] = gt
                    start, _ = calls[ch][cur[ch]]
                    t = (pos[ch] - start) // P
                    pos[ch] += P
                    return gtile[ch][:, t, :]

                gcol = 0
                for w in range(NW):
                    pa = psa.tile([P, H], f32, space="PSUM", tag="pa")
                    first = True
                    if has_bias:
                        diag = soh.tile([P, P], f32, tag="diag")
                        nc.gpsimd.affine_select(
                            out=diag[:],
                            in_=sqdegw[:, w:w + 1].to_broadcast([P, P]),
                            pattern=[[-1, P]], base=0, channel_multiplier=1,
                            compare_op=mybir.AluOpType.is_equal, fill=0.0)
                        nc.tensor.matmul(out=pa[:], lhsT=diag[:],
                                         rhs=bfull[layer][:],
                                         start=True, stop=False)
                        first = False
                    ng_w = int(ngrp[w].sum())
                    done = 0
                    for ch in range(NCH):
                        for g in range(int(ngrp[w, ch])):
                            rhs = next_group(ch)
                            oh = soh.tile([P, P], f32, tag="oh")
                            nc.any.tensor_scalar(
                                out=oh[:], in0=iota_f[:],
                                scalar1=dstslot[:, gcol:gcol + 1],
                                scalar2=None,
                                op0=mybir.AluOpType.is_equal)
                            gcol += 1
                            done += 1
                            nc.tensor.matmul(
                                out=pa[:], lhsT=oh[:], rhs=rhs,
                                start=first, stop=(done == ng_w))
                            first = False

                    # ---- epilogue: relu(dinv * (pa + y_self)) ----
                    ts = sep.tile([P, H], f32, tag="ts")
                    nc.vector.tensor_tensor(
                        out=ts[:], in0=pa[:], in1=ySB[:, w * P:(w + 1) * P],
                        op=mybir.AluOpType.add)
                    s3 = sep.tile([P, H], f32, tag="s3")
                    nc.scalar.activation(
                        out=s3[:], in_=ts[:],
                        func=mybir.ActivationFunctionType.Relu,
                        scale=dinvw[:, w:w + 1])
                    if layer < 2:
                        pt = pst.tile([P, P], f32, space="PSUM", tag="pt")
                        nc.tensor.transpose(out=pt[:], in_=s3[:],
                                            identity=ident[:])
                        nc.vector.tensor_copy(out=hT[:, w * P:(w + 1) * P],
                                              in_=pt[:])
                    else:
                        ohb = soh.tile([P, P], f32, tag="oh")
                        nc.any.tensor_scalar(
                            out=ohb[:], in0=iota_f[:],
                            scalar1=batchslot[:, w:w + 1], scalar2=None,
                            op0=mybir.AluOpType.is_equal)
                        pp = psp.tile([P, H], f32, space="PSUM", tag="pp")
                        nc.tensor.matmul(out=pp[:], lhsT=ohb[:], rhs=s3[:],
                                         start=True, stop=True)
                        if w == 0:
                            nc.vector.tensor_copy(out=pool_acc[:], in_=pp[:])
                        else:
                            nc.vector.tensor_tensor(
                                out=pool_acc[:], in0=pool_acc[:], in1=pp[:],
                                op=mybir.AluOpType.add)

            # ---- pooling finish ----
            nc.sync.dma_start(out=ar_in[:], in_=pool_acc[:])
            nc.gpsimd.collective_compute(
                "AllReduce", mybir.AluOpType.add,
                ins=[ar_in[:]], outs=[ar_out[:]],
                replica_groups=[list(range(NCORES))],
            )
            art = sep.tile([P, H], f32, tag="art")
            nc.sync.dma_start(out=art[:], in_=ar_out[:])
            ptile = sep.tile([P, H], f32, tag="ptile")
            nc.vector.tensor_scalar(
                out=ptile[:], in0=art[:], scalar1=cntinv[:, 0:1],
                scalar2=None, op0=mybir.AluOpType.mult)
            ptp = pst.tile([P, P], f32, space="PSUM", tag="pt")
            nc.tensor.transpose(out=ptp[:], in_=ptile[:], identity=ident[:])
            ptT = sep.tile([P, P], f32, tag="ptT")
            nc.vector.tensor_copy(out=ptT[:], in_=ptp[:])
            po = pst.tile([P, P], f32, space="PSUM", tag="pt")
            nc.tensor.matmul(out=po[:, :C], lhsT=ptT[:], rhs=Wlin[:],
                             start=True, stop=True)
            ot = sep.tile([P, C], f32, tag="ot")
            if has_blin:
                nc.vector.tensor_tensor(out=ot[:], in0=po[:, :C], in1=blin[:],
                                        op=mybir.AluOpType.add)
            else:
                nc.vector.tensor_copy(out=ot[:], in_=po[:, :C])
            nc.sync.dma_start(out=out_d[:], in_=ot[:])

    nc.compile()
    return nc


def kernel(x, edge_index, batch, W0, b0, W1, b1, W2, b2, Wlin, blin):
    x = np.asarray(x, dtype=np.float32)
    batch_np = np.asarray(batch, dtype=np.int64)
    Wl = [np.asarray(w, dtype=np.float32) for w in (W0, W1, W2)]
    bl = [np.asarray(b, dtype=np.float32) for b in (b0, b1, b2)]
    Wlin = np.asarray(Wlin, dtype=np.float32)
    blin = np.asarray(blin, dtype=np.float32)

    ngrp, dinv, deg, cores = _preprocess(np.asarray(edge_index))
    has_bias = any(np.abs(b).max() > 0 for b in bl)
    has_blin = bool(np.abs(blin).max() > 0)

    cnt = np.bincount(batch_np, minlength=G).astype(np.float32)
    cntinv = (1.0 / np.maximum(cnt, 1.0)).astype(np.float32)[:, None]  # [G,1]

    in_maps = []
    for c in range(NCORES):
        widx, dstslot = cores[c]
        lo = c * NP
        xT = np.zeros((P, NPAD), dtype=np.float32)
        xT[:, :NP] = x[lo:lo + NP].T
        dv = np.ones(NPAD, dtype=np.float32)
        dv[:NP] = dinv[lo:lo + NP]
        dinvw = dv.reshape(NW, P).T.copy()
        bs = np.full(NPAD, -1.0, dtype=np.float32)
        bs[:NP] = batch_np[lo:lo + NP].astype(np.float32)
        batchslot = bs.reshape(NW, P).T.copy()
        m = {
            "xT": xT, "widx": widx, "dstslot": dstslot,
            "dinvw": dinvw, "batchslot": batchslot, "cntinv": cntinv,
            "W0": Wl[0], "W1": Wl[1], "W2": Wl[2], "Wlin": Wlin,
        }
        if has_bias:
            sq = np.zeros(NPAD, dtype=np.float32)
            sq[:NP] = np.sqrt(deg[lo:lo + NP])
            m["sqdegw"] = sq.reshape(NW, P).T.copy()
            for i in range(3):
                m[f"Bfull{i}"] = np.tile(bl[i][None, :], (P, 1)).astype(np.float32)
        if has_blin:
            m["blinT"] = np.tile(blin[None, :], (P, 1)).astype(np.float32)
        in_maps.append(m)

    nc = _build(ngrp, has_bias, has_blin)
    res = run_bass_kernel_spmd(nc, in_maps, core_ids=list(range(NCORES)),
                               trace=TRACE)
    global LAST_RESULTS
    LAST_RESULTS = res
    return res.results[0]["out"]


# revision 9
# speedup vs baseline: 5.9044x; 1.0144x over previous
"""Distributed GCN (3x GCNConv + global mean pool + linear) on 8 TRN2 cores.

Sharding: nodes partitioned contiguously across 8 cores; edges partitioned by
dst owner; per-layer node features all-gathered to a full replicated table in
each core's DRAM; per-edge messages fetched with dma_gather; segment-sum over
dst done as one-hot matmuls accumulating in PSUM per 128-node dst window.
"""
import math
import numpy as np
from contextlib import ExitStack

import concourse.bacc as bacc
import concourse.mybir as mybir
from concourse.tile import TileContext
from concourse.bass_utils import run_bass_kernel_spmd
from concourse.masks import make_identity

P = 128
NCORES = 8
N = 100000
E = 1600000
H = 128
C = 10
G = 128
NP = N // NCORES            # 12500 nodes per core
NW = math.ceil(NP / P)      # 98 dst windows per core
NPAD = NW * P               # 12544 padded nodes per core
CHROWS = 25000              # gather chunk rows (int16 limit 32767; 25000 balances buckets)
NCH = math.ceil(N / CHROWS)  # 4
NI = 1024                   # indices per dma_gather call (>1024 crashes SWDGE ring)
NLAYERS = 3                 # debug knob

TRACE = False               # set by test.py for profiling runs
LAST_RESULTS = {}           # debug: per-core raw results


def _wrap_idx(idx):
    """int16 gather index layout: [128, len/16], i -> [i%16, i//16], tiled x8."""
    n = idx.shape[0]
    assert n % 16 == 0
    w = idx.reshape(n // 16, 16).T.astype(np.int16)   # [16, n/16]
    return np.tile(w, (8, 1))                          # [128, n/16]


def _preprocess(edge_index):
    """Partition/sort/pad edges. Returns shared structure + per-core arrays."""
    src = np.asarray(edge_index[0], dtype=np.int64)
    dst = np.asarray(edge_index[1], dtype=np.int64)

    deg = np.bincount(dst, minlength=N).astype(np.float32) + 1.0
    dinv = (1.0 / np.sqrt(deg)).astype(np.float32)

    # self-loop term handled on-device via the ySB slab, not as edges
    src_a = src
    dst_a = dst

    owner = dst_a // NP
    # per-core bucket counts: cnt[core][w][ch]
    w_all = (dst_a - owner * NP) // P
    ch_all = src_a // CHROWS
    flat = ((owner * NW + w_all) * NCH + ch_all).astype(np.int64)
    cnt = np.bincount(flat, minlength=NCORES * NW * NCH).reshape(NCORES, NW, NCH)
    ngrp = np.maximum(np.ceil(cnt.max(axis=0) / P).astype(np.int64), 0)  # [NW, NCH]

    cores = []
    order_all = np.argsort(flat, kind="stable")
    bounds = np.searchsorted(flat[order_all], np.arange(NCORES * NW * NCH + 1))
    for c in range(NCORES):
        idx_parts = [[] for _ in range(NCH)]   # per-chunk local row ids
        slot_parts = []                        # per-group 128 dst slots (f32)
        for w in range(NW):
            for ch in range(NCH):
                b = (c * NW + w) * NCH + ch
                ee = order_all[bounds[b]:bounds[b + 1]]
                k = ee.shape[0]
                npad = int(ngrp[w, ch]) * P - k
                loc = np.zeros(int(ngrp[w, ch]) * P, dtype=np.int64)
                slo = np.full(int(ngrp[w, ch]) * P, -1.0, dtype=np.float32)
                loc[:k] = src_a[ee] - ch * CHROWS
                slo[:k] = (dst_a[ee] - c * NP - w * P).astype(np.float32)
                idx_parts[ch].append(loc)
                slot_parts.append(slo)
        widx = np.concatenate(
            [_wrap_idx(np.concatenate(p)) if p else np.zeros((128, 0), np.int16)
             for p in idx_parts], axis=1)
        dstslot = np.concatenate(slot_parts).reshape(-1, P).T.copy()  # [128, NG]
        cores.append((widx, dstslot))
    return ngrp, dinv, deg, cores


def _build(ngrp, has_bias, has_blin):
    """Build the SPMD bass program (same for all cores)."""
    nc = bacc.Bacc("TRN2", num_devices=NCORES)
    f32 = mybir.dt.float32

    # chunk stream lengths / call table
    L = [int(ngrp[:, ch].sum()) * P for ch in range(NCH)]
    Loff = np.concatenate([[0], np.cumsum(L)])
    NG = int(ngrp.sum())

    # ---- I/O ----
    xT = nc.dram_tensor("xT", [P, NPAD], f32, kind="ExternalInput")
    widx = nc.dram_tensor("widx", [P, Loff[-1] // 16], mybir.dt.int16,
                          kind="ExternalInput")
    dstslot_d = nc.dram_tensor("dstslot", [P, NG], f32, kind="ExternalInput")
    dinvw_d = nc.dram_tensor("dinvw", [P, NW], f32, kind="ExternalInput")
    batchslot_d = nc.dram_tensor("batchslot", [P, NW], f32, kind="ExternalInput")
    cntinv_d = nc.dram_tensor("cntinv", [P, 1], f32, kind="ExternalInput")
    Ws_d = [nc.dram_tensor(f"W{i}", [P, H], f32, kind="ExternalInput")
            for i in range(3)]
    Wlin_d = nc.dram_tensor("Wlin", [P, C], f32, kind="ExternalInput")
    sqdegw_d = bfull_d = None
    if has_bias:
        sqdegw_d = nc.dram_tensor("sqdegw", [P, NW], f32, kind="ExternalInput")
        bfull_d = [nc.dram_tensor(f"Bfull{i}", [P, H], f32, kind="ExternalInput")
                   for i in range(3)]
    blin_d = None
    if has_blin:
        blin_d = nc.dram_tensor("blinT", [P, C], f32, kind="ExternalInput")
    out_d = nc.dram_tensor("out", [G, C], f32, kind="ExternalOutput")

    y_local = nc.dram_tensor("y_local", [NP, H], f32, kind="Internal")
    y_full = nc.dram_tensor("y_full", [N, H], f32, kind="Internal",
                            addr_space="Shared")
    ar_in = nc.dram_tensor("ar_in", [G, H], f32, kind="Internal")
    ar_out = nc.dram_tensor("ar_out", [G, H], f32, kind="Internal",
                            addr_space="Shared")

    with TileContext(nc) as tc:
        with ExitStack() as ctx:
            pers = ctx.enter_context(tc.tile_pool(name="pers", bufs=1))
            sy = ctx.enter_context(tc.tile_pool(name="sy", bufs=3))
            soh = ctx.enter_context(tc.tile_pool(name="soh", bufs=6))
            sep = ctx.enter_context(tc.tile_pool(name="sep", bufs=3))
            sidx = ctx.enter_context(tc.tile_pool(name="sidx", bufs=6))
            gpools = [ctx.enter_context(tc.tile_pool(name=f"gat{ch}", bufs=3))
                      for ch in range(NCH)]
            psy = ctx.enter_context(tc.tile_pool(name="psy", bufs=2, space="PSUM"))
            psa = ctx.enter_context(tc.tile_pool(name="psa", bufs=2, space="PSUM"))
            pst = ctx.enter_context(tc.tile_pool(name="pst", bufs=2, space="PSUM"))
            psp = ctx.enter_context(tc.tile_pool(name="psp", bufs=1, space="PSUM"))

            # ---- persistent tiles ----
            hT = pers.tile([P, NPAD], f32)
            nc.sync.dma_start(out=hT[:], in_=xT[:])
            ySB = pers.tile([P, NPAD], f32)
            dstslot = pers.tile([P, NG], f32)
            nc.sync.dma_start(out=dstslot[:], in_=dstslot_d[:])
            dinvw = pers.tile([P, NW], f32)
            nc.sync.dma_start(out=dinvw[:], in_=dinvw_d[:])
            batchslot = pers.tile([P, NW], f32)
            nc.sync.dma_start(out=batchslot[:], in_=batchslot_d[:])
            cntinv = pers.tile([P, 1], f32)
            nc.sync.dma_start(out=cntinv[:], in_=cntinv_d[:])
            Ws = []
            for i in range(3):
                t = pers.tile([P, H], f32, tag=f"W{i}")
                nc.sync.dma_start(out=t[:], in_=Ws_d[i][:])
                Ws.append(t)
            Wlin = pers.tile([P, C], f32)
            nc.sync.dma_start(out=Wlin[:], in_=Wlin_d[:])
            sqdegw = bfull = blin = None
            if has_bias:
                sqdegw = pers.tile([P, NW], f32)
                nc.sync.dma_start(out=sqdegw[:], in_=sqdegw_d[:])
                bfull = []
                for i in range(3):
                    t = pers.tile([P, H], f32, tag=f"Bf{i}")
                    nc.sync.dma_start(out=t[:], in_=bfull_d[i][:])
                    bfull.append(t)
            if has_blin:
                blin = pers.tile([P, C], f32)
                nc.sync.dma_start(out=blin[:], in_=blin_d[:])

            ident = pers.tile([P, P], f32)
            make_identity(nc, ident[:])
            iota_i = pers.tile([P, P], mybir.dt.int32)
            nc.gpsimd.iota(iota_i[:], pattern=[[1, P]], base=0,
                           channel_multiplier=0)
            iota_f = pers.tile([P, P], f32)
            nc.vector.tensor_copy(out=iota_f[:], in_=iota_i[:])

            pool_acc = pers.tile([P, H], f32)   # pooled [graph, feat] accum
            nc.vector.memset(pool_acc[:], 0.0)

            # gather call schedule per chunk: list of (start, n) within chunk
            calls = []
            for ch in range(NCH):
                cs = []
                p = 0
                while p < L[ch]:
                    n = min(NI, L[ch] - p)
                    cs.append((p, n))
                    p += n
                calls.append(cs)

            for layer in range(NLAYERS):
                # ---- y = (h @ W) * dinv per window -> y_local -> allgather
                for w in range(NW):
                    py = psy.tile([P, H], f32, space="PSUM", tag="py")
                    nc.tensor.matmul(out=py[:], lhsT=hT[:, w * P:(w + 1) * P],
                                     rhs=Ws[layer][:], start=True, stop=True)
                    nc.vector.tensor_scalar(
                        out=ySB[:, w * P:(w + 1) * P], in0=py[:],
                        scalar1=dinvw[:, w:w + 1],
                        scalar2=None, op0=mybir.AluOpType.mult)
                    rows = min(NP - w * P, P)
                    nc.sync.dma_start(out=y_local[w * P:w * P + rows, :],
                                      in_=ySB[:rows, w * P:(w + 1) * P])
                nc.gpsimd.collective_compute(
                    "AllGather", mybir.AluOpType.bypass,
                    ins=[y_local[:]], outs=[y_full[:]],
                    replica_groups=[list(range(NCORES))],
                )

                # ---- edge gather + segment-sum matmuls ----
                # stream state per chunk
                cur = [-1] * NCH          # current call index per chunk
                gtile = [None] * NCH
                pos = [0] * NCH           # consumed edges within chunk

                def next_group(ch):
                    if cur[ch] < 0 or pos[ch] >= calls[ch][cur[ch]][0] + calls[ch][cur[ch]][1]:
                        cur[ch] += 1
                        start, n = calls[ch][cur[ch]]
                        it = sidx.tile([P, NI // 16], mybir.dt.int16, tag="idx")
                        c0 = (Loff[ch] + start) // 16
                        nc.sync.dma_start(out=it[:, :n // 16],
                                          in_=widx[:, c0:c0 + n // 16])
                        gt = gpools[ch].tile([P, NI // P, H], f32, tag=f"g{ch}")
                        rows0 = ch * CHROWS
                        rows1 = min(rows0 + CHROWS, N)
                        nc.gpsimd.dma_gather(
                            out_ap=gt[:, :n // P, :],
                            in_ap=y_full[rows0:rows1],
                            idxs_ap=it[:, :n // 16],
                            num_idxs=n,
                            num_idxs_reg=n,
                            elem_size=H,
                        )
                        gtile[ch] = gt
                    start, _ = calls[ch][cur[ch]]
                    t = (pos[ch] - start) // P
                    pos[ch] += P
                    return gtile[ch][:, t, :]

                gcol = 0
                for w in range(NW):
                    pa = psa.tile([P, H], f32, space="PSUM", tag="pa")
                    first = True
                    if has_bias:
                        diag = soh.tile([P, P], f32, tag="diag")
                        nc.gpsimd.affine_select(
                            out=diag[:],
                            in_=sqdegw[:, w:w + 1].to_broadcast([P, P]),
                            pattern=[[-1, P]], base=0, channel_multiplier=1,
                            compare_op=mybir.AluOpType.is_equal, fill=0.0)
                        nc.tensor.matmul(out=pa[:], lhsT=diag[:],
                                         rhs=bfull[layer][:],
                                         start=True, stop=False)
                        first = False
                    ng_w = int(ngrp[w].sum())
                    done = 0
                    for ch in range(NCH):
                        for g in range(int(ngrp[w, ch])):
                            rhs = next_group(ch)
                            oh = soh.tile([P, P], f32, tag="oh")
                            nc.any.tensor_scalar(
                                out=oh[:], in0=iota_f[:],
                                scalar1=dstslot[:, gcol:gcol + 1],
                                scalar2=None,
                                op0=mybir.AluOpType.is_equal)
                            gcol += 1
                            done += 1
                            nc.tensor.matmul(
                                out=pa[:], lhsT=oh[:], rhs=rhs,
                                start=first, stop=(done == ng_w))
                            first = False

                    # ---- epilogue: relu(dinv * (pa + y_self)) ----
                    ts = sep.tile([P, H], f32, tag="ts")
                    nc.vector.tensor_tensor(
                        out=ts[:], in0=pa[:], in1=ySB[:, w * P:(w + 1) * P],
                        op=mybir.AluOpType.add)
                    s3 = sep.tile([P, H], f32, tag="s3")
                    nc.scalar.activation(
                        out=s3[:], in_=ts[:],
                        func=mybir.ActivationFunctionType.Relu,
                        scale=dinvw[:, w:w + 1])
                    if layer < 2:
                        pt = pst.tile([P, P], f32, space="PSUM", tag="pt")
                        nc.tensor.transpose(out=pt[:], in_=s3[:],
                                            identity=ident[:])
                        nc.vector.tensor_copy(out=hT[:, w * P:(w + 1) * P],
                                              in_=pt[:])
                    else:
                        ohb = soh.tile([P, P], f32, tag="oh")
                        nc.any.tensor_scalar(
                            out=ohb[:], in0=iota_f[:],
                            scalar1=batchslot[:, w:w + 1], scalar2=None,
                            op0=mybir.AluOpType.is_equal)
                        pp = psp.tile([P, H], f32, space="PSUM", tag="pp")
                        nc.tensor.matmul(out=pp[:], lhsT=ohb[:], rhs=s3[:],
                                         start=True, stop=True)
                        if w == 0:
                            nc.vector.tensor_copy(out=pool_acc[:], in_=pp[:])
                        else:
                            nc.vector.tensor_tensor(
                                out=pool_acc[:], in0=pool_acc[:], in1=pp[:],
                                op=mybir.AluOpType.add)

            # ---- pooling finish ----
            nc.sync.dma_start(out=ar_in[:], in_=pool_acc[:])
            nc.gpsimd.collective_compute(
                "AllReduce", mybir.AluOpType.add,
                ins=[ar_in[:]], outs=[ar_out[:]],
                replica_groups=[list(range(NCORES))],
            )
            art = sep.tile([P, H], f32, tag="art")
            nc.sync.dma_start(out=art[:], in_=ar_out[:])
            ptile = sep.tile([P, H], f32, tag="ptile")
            nc.vector.tensor_scalar(
                out=ptile[:], in0=art[:], scalar1=cntinv[:, 0:1],
                scalar2=None, op0=mybir.AluOpType.mult)
            ptp = pst.tile([P, P], f32, space="PSUM", tag="pt")
            nc.tensor.transpose(out=ptp[:], in_=ptile[:], identity=ident[:])
            ptT = sep.tile([P, P], f32, tag="ptT")
            nc.vector.tensor_copy(out=ptT[:], in_=ptp[:])
            po = pst.tile([P, P], f32, space="PSUM", tag="pt")
            nc.tensor.matmul(out=po[:, :C], lhsT=ptT[:], rhs=Wlin[:],
                             start=True, stop=True)
            ot = sep.tile([P, C], f32, tag="ot")
            if has_blin:
                nc.vector.tensor_tensor(out=ot[:], in0=po[:, :C], in1=blin[:],
                                        op=mybir.AluOpType.add)
            else:
                nc.vector.tensor_copy(out=ot[:], in_=po[:, :C])
            nc.sync.dma_start(out=out_d[:], in_=ot[:])

    nc.compile()
    return nc


def kernel(x, edge_index, batch, W0, b0, W1, b1, W2, b2, Wlin, blin):
    x = np.asarray(x, dtype=np.float32)
    batch_np = np.asarray(batch, dtype=np.int64)
    Wl = [np.asarray(w, dtype=np.float32) for w in (W0, W1, W2)]
    bl = [np.asarray(b, dtype=np.float32) for b in (b0, b1, b2)]
    Wlin = np.asarray(Wlin, dtype=np.float32)
    blin = np.asarray(blin, dtype=np.float32)

    ngrp, dinv, deg, cores = _preprocess(np.asarray(edge_index))
    has_bias = any(np.abs(b).max() > 0 for b in bl)
    has_blin = bool(np.abs(blin).max() > 0)

    cnt = np.bincount(batch_np, minlength=G).astype(np.float32)
    cntinv = (1.0 / np.maximum(cnt, 1.0)).astype(np.float32)[:, None]  # [G,1]

    in_maps = []
    for c in range(NCORES):
        widx, dstslot = cores[c]
        lo = c * NP
        xT = np.zeros((P, NPAD), dtype=np.float32)
        xT[:, :NP] = x[lo:lo + NP].T
        dv = np.ones(NPAD, dtype=np.float32)
        dv[:NP] = dinv[lo:lo + NP]
        dinvw = dv.reshape(NW, P).T.copy()
        bs = np.full(NPAD, -1.0, dtype=np.float32)
        bs[:NP] = batch_np[lo:lo + NP].astype(np.float32)
        batchslot = bs.reshape(NW, P).T.copy()
        m = {
            "xT": xT, "widx": widx, "dstslot": dstslot,
            "dinvw": dinvw, "batchslot": batchslot, "cntinv": cntinv,
            "W0": Wl[0], "W1": Wl[1], "W2": Wl[2], "Wlin": Wlin,
        }
        if has_bias:
            sq = np.zeros(NPAD, dtype=np.float32)
            sq[:NP] = np.sqrt(deg[lo:lo + NP])
            m["sqdegw"] = sq.reshape(NW, P).T.copy()
            for i in range(3):
                m[f"Bfull{i}"] = np.tile(bl[i][None, :], (P, 1)).astype(np.float32)
        if has_blin:
            m["blinT"] = np.tile(blin[None, :], (P, 1)).astype(np.float32)
        in_maps.append(m)

    nc = _build(ngrp, has_bias, has_blin)
    res = run_bass_kernel_spmd(nc, in_maps, core_ids=list(range(NCORES)),
                               trace=TRACE)
    global LAST_RESULTS
    LAST_RESULTS = res
    return res.results[0]["out"]
